# revision 1
# baseline (speedup 1.0000x reference)
"""Mamba block kernel for 8 Trainium2 NeuronCores.

Sharding: core c handles batch c//2 and d_inner half c%2 (D_loc=1024).
Within each pair (same batch): x_proj partials are pair-AllReduced per
time-half (96x1024), out_proj partials pair-ReduceScattered per
time-chunk (each core emits half of d_model).

Pipeline is split into two time halves so the in_proj/conv GEMMs of
half 1 overlap the selective scan of half 0.

Selective scan: DVE tensor_tensor_scan along time (free dim), d_inner on
partitions, one scan per (d_tile, n, t_chunk); decay a = exp(-A*dt) on
ACT with per-partition scale = -A[:, j, n]; in_proj/conv in bf16, other
GEMMs fp32r; the n-reduction sum_n C*h and the u*D skip ride PE PSUM
accumulation via identity / diag matmuls.
"""
import sys
sys.path.insert(0, "/opt/trn_rl_repo")
import numpy as np
import ml_dtypes
import concourse.bass as bass
import concourse.bacc as bacc
import concourse.mybir as mybir
from concourse.tile import TileContext
from concourse.bass_utils import run_bass_kernel_spmd

F32 = mybir.dt.float32
F32R = mybir.dt.float32r
BF16 = mybir.dt.bfloat16
OP = mybir.AluOpType
AF = mybir.ActivationFunctionType

B_, L, DM = 4, 2048, 1024       # batch, seqlen, d_model
DI = 2048                        # d_inner (global)
DL = 1024                        # d_inner per core
N = 16                           # d_state
RK = 64                          # dt_rank
KC = 4                           # conv width
TC = 512                         # time chunk
HL = L // 2                      # half length (1024)
NCH_H = HL // TC                 # chunks per half (2)
NJ = DL // 128                   # 8 d-tiles per core
NK = DM // 128                   # 8 k-tiles over d_model
NM = DM // 128                   # 8 out d_model tiles
PAIRS = [[0, 1], [2, 3], [4, 5], [6, 7]]

_CACHED_NC = {}


def _build(reps=1):
    nc = bacc.Bacc(num_devices=8)

    # ---- parameters (per-core shards) ----
    hst = nc.declare_dram_parameter("hst", [DM, L], BF16, isOutput=False)
    wxT = nc.declare_dram_parameter("wxT", [DM, DL], BF16, isOutput=False)
    wzT = nc.declare_dram_parameter("wzT", [DM, DL], BF16, isOutput=False)
    convd = nc.declare_dram_parameter("convd", [NJ, KC, 128, 128], BF16,
                                      isOutput=False)
    convb = nc.declare_dram_parameter("convb", [128, NJ], F32, isOutput=False)
    wxpT = nc.declare_dram_parameter("wxpT", [DL, RK + 2 * N], F32, isOutput=False)
    wdtT = nc.declare_dram_parameter("wdtT", [RK, DL], F32, isOutput=False)
    bdt = nc.declare_dram_parameter("bdt", [128, NJ], F32, isOutput=False)
    woT = nc.declare_dram_parameter("woT", [DL, DM], F32, isOutput=False)
    alog = nc.declare_dram_parameter("alog", [128, NJ * N], F32, isOutput=False)
    diagd = nc.declare_dram_parameter("diagd", [NJ, 128, 128], BF16, isOutput=False)
    ident = nc.declare_dram_parameter("ident", [128, 128], BF16, isOutput=False)
    sel = nc.declare_dram_parameter("sel", [N, N * 128], BF16, isOutput=False)
    oslab = nc.declare_dram_parameter("oslab", [DM // 2, L], F32, isOutput=True)

    P = dict(hst=hst, wxT=wxT, wzT=wzT, convd=convd, convb=convb, wxpT=wxpT,
             wdtT=wdtT, bdt=bdt, woT=woT, alog=alog, diagd=diagd, ident=ident,
             sel=sel, oslab=oslab)

    with TileContext(nc) as tc:
        with (
            tc.tile_pool(name="const", bufs=1) as cp,
        ):
            C = {}
            C["convb"] = cp.tile([128, NJ], F32, tag="convb", name="convb_t")
            nc.sync.dma_start(out=C["convb"][:, :], in_=convb[:, :])
            C["bdt"] = cp.tile([128, NJ], F32, tag="bdt", name="bdt_t")
            nc.sync.dma_start(out=C["bdt"][:, :], in_=bdt[:, :])
            alog_t = cp.tile([128, NJ * N], F32, tag="alog")
            nc.sync.dma_start(out=alog_t[:, :], in_=alog[:, :])
            C["sel"] = cp.tile([N, N * 128], BF16, tag="sel", name="sel_t")
            nc.sync.dma_start(out=C["sel"][:, :], in_=sel[:, :])
            C["wxpT"] = cp.tile([128, NK, RK + 2 * N], F32R, tag="wxpT",
                                name="wxpT_t")
            nc.sync.dma_start(
                out=C["wxpT"][:, :, :],
                in_=wxpT[:, :].rearrange("(k p) w -> p k w", k=NK).bitcast(F32R))
            C["ident"] = cp.tile([128, 128], BF16, tag="ident", name="ident_t")
            nc.sync.dma_start(out=C["ident"][:, :], in_=ident[:, :])
            C["diagd"] = cp.tile([128, NJ, 128], BF16, tag="diagd", name="diagd_t")
            nc.sync.dma_start(
                out=C["diagd"][:, :, :],
                in_=diagd[:, :, :].rearrange("j p q -> p j q"))
            C["wdtT"] = cp.tile([RK, DL], F32R, tag="wdtT", name="wdtT_t")
            nc.sync.dma_start(out=C["wdtT"][:, :], in_=wdtT[:, :].bitcast(F32R))
            negA = cp.tile([128, NJ * N], F32, tag="negA")
            nc.scalar.activation(negA[:, :], alog_t[:, :], AF.Exp)
            nc.vector.tensor_scalar_mul(negA[:, :], negA[:, :], -1.0)
            C["negA"] = negA
            C["carries"] = [cp.tile([128, N], F32, tag=f"carry{j}",
                                    name=f"carry{j}") for j in range(NJ)]
            C["tails"] = cp.tile([128, NJ, KC - 1], BF16, tag="tails",
                                 name="tails_t")
            zero3 = cp.tile([128, 4], BF16, tag="zero3")
            nc.vector.memset(zero3[:, :], 0.0)
            C["zero3"] = zero3
            # woT in bf16 (resident for out_proj)
            woT_b = []
            with tc.tile_pool(name="wofp", bufs=2) as wofp:
                for j in range(NJ):
                    wof = wofp.tile([128, DM], F32, tag="wof", name="wof")
                    nc.sync.dma_start(out=wof[:, :],
                                      in_=woT[j * 128:(j + 1) * 128, :])
                    wob = cp.tile([128, DM], BF16, tag=f"wo{j}", name=f"wo{j}")
                    nc.vector.tensor_copy(wob[:, :], wof[:, :])
                    woT_b.append(wob)
            C["woT_b"] = woT_b

            for rep in range(reps):
                D_ = {}
                D_["xdbl_in"] = [
                    nc.dram_tensor(f"xdbl_in{rep}_{h}", [RK + 2 * N, HL], F32)
                    for h in range(2)]
                D_["xdbl_out"] = [
                    nc.dram_tensor(f"xdbl_out{rep}_{h}", [RK + 2 * N, HL], F32)
                    for h in range(2)]
                D_["g_dram"] = nc.dram_tensor(f"g_dram{rep}", [DL, L], BF16)
                D_["u_dram"] = nc.dram_tensor(f"u_dram{rep}", [DL, L], BF16)
                D_["oc_in"] = [nc.dram_tensor(f"oc_in{rep}_{c}", [DM, TC], F32)
                               for c in range(L // TC)]
                D_["oc_out"] = [nc.dram_tensor(f"oc_out{rep}_{c}",
                                               [DM // 2, TC], F32)
                                for c in range(L // TC)]
                _emit_p1_half(nc, tc, P, C, D_, 0)
                _emit_scan_half(nc, tc, P, C, D_, 0)
                _emit_p1_half(nc, tc, P, C, D_, 1)
                _emit_scan_half(nc, tc, P, C, D_, 1)

    nc.finalize()
    return nc


def _emit_ar(nc, D_, h):
    nc.gpsimd.collective_compute(
        "AllReduce", OP.add, replica_groups=PAIRS,
        ins=[D_["xdbl_in"][h][:, :]], outs=[D_["xdbl_out"][h][:, :]],
    )


def _emit_p1_half(nc, tc, P, C, D_, h):
    """in_proj (bf16) + conv + x_proj partials + u spill for half h; then
    the pair AllReduce; then the z/gate GEMMs (overlap the AR + scan start)."""
    t0 = h * HL
    with (
        tc.tile_pool(name=f"hsTp{h}", bufs=1) as hp,
        tc.tile_pool(name=f"wst{h}", bufs=4) as wp,
        tc.tile_pool(name=f"xc{h}", bufs=2) as xcp,
        tc.tile_pool(name=f"ztmp{h}", bufs=2) as zp,
        tc.tile_pool(name=f"up{h}", bufs=2) as upool,
        tc.tile_pool(name=f"ps1{h}", bufs=2, space="PSUM") as psA,
    ):
        hsT = []
        for k in range(NK):
            t = hp.tile([128, HL], BF16, tag=f"hsT{k}", name=f"hsT{k}")
            nc.sync.dma_start(
                out=t[:, :], in_=P["hst"][k * 128:(k + 1) * 128, t0:t0 + HL])
            hsT.append(t)

        psx = [psA.tile([RK + 2 * N, TC], F32, tag="psx", name=f"psx{c}")
               for c in range(NCH_H)]

        for j in range(NJ):
            wt = wp.tile([128, NK, 128], BF16, tag="w_in", name="w_in")
            nc.sync.dma_start(
                out=wt[:, :, :],
                in_=P["wxT"][:, j * 128:(j + 1) * 128].rearrange(
                    "(k p) q -> p k q", k=NK))
            dconv = wp.tile([128, KC, 128], BF16, tag="w_cv", name="w_cv")
            nc.sync.dma_start(
                out=dconv[:, :, :],
                in_=P["convd"][j, :, :, :].rearrange("k p q -> p k q"))

            xcj = xcp.tile([128, KC - 1 + HL], BF16, tag="xcj", name="xcj")
            if h == 0:
                nc.vector.tensor_copy(xcj[:, 0:KC - 1], C["zero3"][:, 0:KC - 1])
            else:
                nc.vector.tensor_copy(xcj[:, 0:KC - 1], C["tails"][:, j, :])

            for c in range(NCH_H):
                csl_g = slice(t0 + c * TC, t0 + (c + 1) * TC)
                ps = psA.tile([128, TC], F32, tag="ps", name="ps")
                for k in range(NK):
                    nc.tensor.matmul(
                        ps[:, :], wt[:, k, :], hsT[k][:, c * TC:(c + 1) * TC],
                        start=(k == 0), stop=(k == NK - 1))
                nc.vector.tensor_scalar(
                    xcj[:, KC - 1 + c * TC: KC - 1 + (c + 1) * TC],
                    ps[:, :], 0.0, 1.0, op0=OP.max, op1=OP.min)
                psc = psA.tile([128, TC], F32, tag="ps", name="psc")
                for k in range(KC):
                    nc.tensor.matmul(
                        psc[:, :], dconv[:, k, :],
                        xcj[:, c * TC + k: c * TC + k + TC],
                        start=(k == 0), stop=(k == KC - 1))
                us0 = upool.tile([128, TC], F32, tag="us0", name="us0")
                nc.scalar.activation(us0[:, :], psc[:, :], AF.Silu,
                                     bias=C["convb"][:, j:j + 1])
                uc = upool.tile([128, TC], F32R, tag="uc", name="uc")
                nc.vector.tensor_scalar(uc[:, :], us0[:, :], 0.0, 1.0,
                                        op0=OP.max, op1=OP.min)
                ubc = upool.tile([128, TC], BF16, tag="ubc", name="ubc")
                nc.vector.tensor_copy(ubc[:, :], uc[:, :].bitcast(F32))
                nc.sync.dma_start(
                    out=D_["u_dram"][j * 128:(j + 1) * 128, csl_g],
                    in_=ubc[:, :])
                nc.tensor.matmul(
                    psx[c][:, :], C["wxpT"][:, j, :], uc[:, :],
                    start=(j == 0), stop=(j == NJ - 1))

            if h == 0:
                nc.vector.tensor_copy(C["tails"][:, j, :], xcj[:, HL:HL + KC - 1])

        for c in range(NCH_H):
            cps = zp.tile([RK + 2 * N, TC], F32, tag="xdblc", name="xdblc")
            nc.scalar.copy(cps[:, :], psx[c][:, :])
            nc.sync.dma_start(out=D_["xdbl_in"][h][:, c * TC:(c + 1) * TC],
                              in_=cps[:, :])
        _emit_ar(nc, D_, h)

        # z / gate GEMMs (after the AR so the scan can start meanwhile)
        for j in range(NJ):
            wt = wp.tile([128, NK, 128], BF16, tag="w_in", name="w_inz")
            nc.sync.dma_start(
                out=wt[:, :, :],
                in_=P["wzT"][:, j * 128:(j + 1) * 128].rearrange(
                    "(k p) q -> p k q", k=NK))
            for c in range(NCH_H):
                csl_g = slice(t0 + c * TC, t0 + (c + 1) * TC)
                psz = psA.tile([128, TC], F32, tag="ps", name="psz")
                for k in range(NK):
                    nc.tensor.matmul(
                        psz[:, :], wt[:, k, :], hsT[k][:, c * TC:(c + 1) * TC],
                        start=(k == 0), stop=(k == NK - 1))
                zt = zp.tile([128, TC], F32, tag="zt", name="zt")
                nc.vector.tensor_scalar(zt[:, :], psz[:, :], 0.0, 1.0,
                                        op0=OP.max, op1=OP.min)
                gt = zp.tile([128, TC], BF16, tag="gt", name="gt")
                nc.scalar.activation(gt[:, :], zt[:, :], AF.Silu)
                nc.sync.dma_start(
                    out=D_["g_dram"][j * 128:(j + 1) * 128, csl_g],
                    in_=gt[:, :])


def _emit_scan_half(nc, tc, P, C, D_, h):
    """scan + gate + out_proj for time half h (chunks of TC)."""
    t0 = h * HL
    carries = C["carries"]
    negA = C["negA"]
    with (
        tc.tile_pool(name=f"rb{h}", bufs=1) as rbp,
        tc.tile_pool(name=f"bc{h}", bufs=1) as bcp,
        tc.tile_pool(name=f"sc{h}", bufs=2) as scp,
        tc.tile_pool(name=f"aab{h}", bufs=4) as ap_,
        tc.tile_pool(name=f"hp5{h}", bufs=1) as hp5,
        tc.tile_pool(name=f"tr{h}", bufs=2) as trp,
        tc.tile_pool(name=f"g5{h}", bufs=3) as g5,
        tc.tile_pool(name=f"yg{h}", bufs=8) as ygp,
        tc.tile_pool(name=f"ps5{h}", bufs=1, space="PSUM") as ps5,
        tc.tile_pool(name=f"pmix{h}", bufs=2, space="PSUM") as psy5,
        tc.tile_pool(name=f"pso5{h}", bufs=1, space="PSUM") as pso5,
    ):
        xdbl_out = D_["xdbl_out"][h]
        dtraw = rbp.tile([RK, HL], F32R, tag="dtraw", name="dtraw")
        Brow = rbp.tile([N, HL], BF16, tag="Brow", name="Brow")
        Crow = rbp.tile([N, HL], BF16, tag="Crow", name="Crow")
        with tc.tile_pool(name=f"rbt{h}", bufs=1) as rbt:
            dtraw_f = rbt.tile([RK, HL], F32, tag="dtrawf", name="dtrawf")
            nc.sync.dma_start(out=dtraw_f[:, :], in_=xdbl_out[0:RK, :])
            nc.vector.tensor_scalar(dtraw[:, :], dtraw_f[:, :], 0.0, 1.0,
                                    op0=OP.max, op1=OP.min)
            browf = rbt.tile([N, HL], F32, tag="browf", name="browf")
            nc.sync.dma_start(out=browf[:, :], in_=xdbl_out[RK:RK + N, :])
            nc.vector.tensor_copy(Brow[:, :], browf[:, :])
            crowf = rbt.tile([N, HL], F32, tag="crowf", name="crowf")
            nc.sync.dma_start(out=crowf[:, :], in_=xdbl_out[RK + N:RK + 2 * N, :])
            nc.vector.tensor_copy(Crow[:, :], crowf[:, :])

        for c in range(NCH_H):
            gc = h * NCH_H + c            # global chunk index
            csl = slice(c * TC, (c + 1) * TC)          # within-half slice
            csl_g = slice(t0 + c * TC, t0 + (c + 1) * TC)  # global slice
            # B/C broadcast for this chunk: [128, N, TC] bf16
            bcast = {}
            for name, row in (("B", Brow), ("C", Crow)):
                dest = bcp.tile([128, N, TC], BF16, tag=f"bc{name}",
                                name=f"bc{name}")
                for n in range(N):
                    pb = psy5.tile([128, TC], F32, tag="pmix", name="psbc")
                    nc.tensor.matmul(
                        pb[:, :], C["sel"][:, n * 128:(n + 1) * 128],
                        row[:, csl], start=True, stop=True)
                    nc.scalar.copy(dest[:, n, :], pb[:, :])
                bcast[name] = dest

            ygs = []
            for j in range(NJ):
                # dt_proj -> softplus -> clip
                psd = ps5.tile([128, TC], F32, tag="psd", name="psd")
                nc.tensor.matmul(
                    psd[:, :], C["wdtT"][:, j * 128:(j + 1) * 128],
                    dtraw[:, csl], start=True, stop=True)
                spe = scp.tile([128, TC], F32, tag="spe", name="spe")
                nc.scalar.activation(spe[:, :], psd[:, :], AF.Exp,
                                     bias=C["bdt"][:, j:j + 1])
                dt = scp.tile([128, TC], F32, tag="dt", name="dt")
                nc.scalar.activation(dt[:, :], spe[:, :], AF.Ln, bias=1.0)
                nc.vector.tensor_scalar(dt[:, :], dt[:, :], 1e-4, 20.0,
                                        op0=OP.max, op1=OP.min)
                # u / g readbacks
                urb = g5.tile([128, TC], BF16, tag="urb", name="urb")
                nc.scalar.dma_start(
                    out=urb[:, :], in_=D_["u_dram"][j * 128:(j + 1) * 128, csl_g])
                grb = g5.tile([128, TC], BF16, tag="grb", name="grb")
                nc.scalar.dma_start(
                    out=grb[:, :], in_=D_["g_dram"][j * 128:(j + 1) * 128, csl_g])
                # dtu (bf16)
                dtu = scp.tile([128, TC], BF16, tag="dtu", name="dtu")
                nc.vector.tensor_tensor(
                    out=dtu[:, :], in0=dt[:, :], in1=urb[:, :], op=OP.mult)

                psy = psy5.tile([128, TC], F32, tag="pmix", name="psy")
                ht_halves = []
                for nh in range(2):
                    # b for this n-half (second half on GPSIMD to offload DVE)
                    bt = trp.tile([128, 8, TC], BF16, tag="btch", name="bt")
                    nc.vector.tensor_tensor(
                        out=bt[:, :, :],
                        in0=dtu[:, None, :].broadcast_to([128, 8, TC]),
                        in1=bcast["B"][:, 8 * nh:8 * nh + 8, :], op=OP.mult)
                    ht = hp5.tile([128, 8, TC], BF16, tag="ht", bufs=2,
                                  name="ht")
                    for nn in range(8):
                        n = 8 * nh + nn
                        an = ap_.tile([128, TC], F32, tag="an", name="an")
                        nc.scalar.activation(
                            an[:, :], dt[:, :], AF.Exp,
                            scale=negA[:, j * N + n: j * N + n + 1])
                        init = 0.0 if gc == 0 else carries[j][:, n:n + 1]
                        nc.vector.tensor_tensor_scan(
                            ht[:, nn, :], an[:, :], bt[:, nn, :], init,
                            op0=OP.mult, op1=OP.add)
                    nc.vector.tensor_copy(
                        carries[j][:, 8 * nh:8 * nh + 8], ht[:, :, TC - 1])
                    # CH = h * C_bcast (reuses bt slot pool tag)
                    # (NOTE: gpsimd.tensor_tensor crashes the device -- keep DVE)
                    ch = trp.tile([128, 8, TC], BF16, tag="btch", name="ch")
                    nc.vector.tensor_tensor(
                        out=ch[:, :, :], in0=ht[:, :, :],
                        in1=bcast["C"][:, 8 * nh:8 * nh + 8, :], op=OP.mult)
                    # PE: accumulate sum_n ch into psy
                    for nn in range(8):
                        nc.tensor.matmul(
                            psy[:, :], C["ident"][:, :], ch[:, nn, :],
                            start=(nh == 0 and nn == 0), stop=False)
                    ht_halves.append(ht)
                # skip term
                nc.tensor.matmul(
                    psy[:, :], C["diagd"][:, j, :], urb[:, :],
                    start=False, stop=True)
                # clip -> bf16 y ; gate
                yt = g5.tile([128, TC], BF16, tag="yt", name="yt")
                nc.vector.tensor_scalar(yt[:, :], psy[:, :], 0.0, 1.0,
                                        op0=OP.max, op1=OP.min)
                yg = ygp.tile([128, TC], BF16, tag="yg", name="yg")
                nc.vector.tensor_tensor(
                    out=yg[:, :], in0=yt[:, :], in1=grb[:, :], op=OP.mult)
                ygs.append(yg)

            # out_proj for this chunk (bf16), partials -> RS
            for m in range(NM):
                pso = pso5.tile([128, TC], F32, tag="pso", name="pso")
                for j in range(NJ):
                    nc.tensor.matmul(
                        pso[:, :], C["woT_b"][j][:, m * 128:(m + 1) * 128],
                        ygs[j][:, :], start=(j == 0), stop=(j == NJ - 1))
                osb = g5.tile([128, TC], F32, tag="osb", bufs=2, name="osb")
                nc.scalar.copy(osb[:, :], pso[:, :])
                nc.scalar.dma_start(
                    out=D_["oc_in"][gc][m * 128:(m + 1) * 128, :], in_=osb[:, :])
            nc.gpsimd.collective_compute(
                "ReduceScatter", OP.add, replica_groups=PAIRS,
                ins=[D_["oc_in"][gc][:, :]], outs=[D_["oc_out"][gc][:, :]],
            )
            nc.gpsimd.dma_start(out=P["oslab"][:, csl_g], in_=D_["oc_out"][gc][:, :])


def _shard(inputs):
    hs = np.asarray(inputs["hidden_states"], np.float32)
    W_in = np.asarray(inputs["W_in"], np.float32)
    conv_w = np.asarray(inputs["conv_w"], np.float32)
    conv_b = np.asarray(inputs["conv_b"], np.float32)
    W_x = np.asarray(inputs["W_x"], np.float32)
    W_dt = np.asarray(inputs["W_dt"], np.float32)
    b_dt = np.asarray(inputs["b_dt"], np.float32)
    W_out = np.asarray(inputs["W_out"], np.float32)
    A_log = np.asarray(inputs["A_log"], np.float32)
    D = np.asarray(inputs["D"], np.float32)
    bf = ml_dtypes.bfloat16

    sel = np.zeros((N, N * 128), bf)
    for n in range(N):
        sel[n, n * 128:(n + 1) * 128] = 1.0
    ident = np.eye(128, dtype=bf)

    in_maps = []
    idx = np.arange(128)
    for c in range(8):
        b, dh = c // 2, c % 2
        dsl = slice(dh * DL, (dh + 1) * DL)
        conv_w_l = conv_w[dsl, 0, :]                      # (DL, KC)
        convd = np.zeros((NJ, KC, 128, 128), bf)
        for j in range(NJ):
            for k in range(KC):
                convd[j, k, idx, idx] = conv_w_l[j * 128 + idx, k].astype(bf)
        diagd = np.zeros((NJ, 128, 128), bf)
        for j in range(NJ):
            diagd[j, idx, idx] = D[dsl][j * 128 + idx].astype(bf)
        m = {
            "hst": np.ascontiguousarray(hs[b].T).astype(bf),
            "wxT": np.ascontiguousarray(W_in[dsl].T).astype(bf),
            "wzT": np.ascontiguousarray(
                W_in[DI + dh * DL: DI + (dh + 1) * DL].T).astype(bf),
            "convd": convd,
            "convb": np.ascontiguousarray(conv_b[dsl].reshape(NJ, 128).T),
            "wxpT": np.ascontiguousarray(W_x[:, dsl].T),
            "wdtT": np.ascontiguousarray(W_dt[dsl].T),
            "bdt": np.ascontiguousarray(b_dt[dsl].reshape(NJ, 128).T),
            "woT": np.ascontiguousarray(W_out[:, dsl].T),
            "alog": np.ascontiguousarray(
                A_log[dsl].reshape(NJ, 128, N).transpose(1, 0, 2).reshape(128, NJ * N)),
            "diagd": diagd,
            "ident": ident,
            "sel": sel,
        }
        in_maps.append(m)
    return in_maps


def kernel(**inputs):
    if 1 not in _CACHED_NC:
        _CACHED_NC[1] = _build(1)
    nc = _CACHED_NC[1]
    in_maps = _shard(inputs)
    res = run_bass_kernel_spmd(nc, in_maps, core_ids=list(range(8)))
    out = np.empty((B_, L, DM), np.float32)
    for b in range(B_):
        s0 = res.results[2 * b]["oslab"]       # (512, L): d_model rows 0:512
        s1 = res.results[2 * b + 1]["oslab"]   # (512, L): d_model rows 512:1024
        out[b] = np.concatenate([s0, s1], axis=0).T
    return out



# revision 11
# speedup vs baseline: 4.5676x; 4.5676x over previous
"""Mamba block kernel for 8 Trainium2 NeuronCores.

Sharding: core c handles batch c//2 and d_inner half c%2 (DL=1024).
x_proj partials are pair-AllReduced ([96,L] f32); out_proj partials are
pair-ReduceScattered ([DM,L] bf16, each core emits half of d_model).

This environment steps instructions at ~30-40us each regardless of size,
so the kernel minimizes INSTRUCTION COUNT:
- all GEMMs in f32r (f32r matmuls emit no separate LDWEIGHTS),
- the 16 scan states per d-tile run as ONE tensor_tensor_scan over the
  flattened (n, t) free dim, with a zero-decay flush column per segment
  that both resets the state and injects the chunk carry,
- the sum over n (y_t = sum_n C h + u D) is a SECOND add-scan over an
  (t, n)-major buffer with a flush slot, so it is 1 instruction too,
- B/C rows are partition-broadcast with a single stride-0 DMA,
- conv1d is 7 DVE shift-mac ops per d-tile instead of PE diag matmuls.
"""
import sys
sys.path.insert(0, "/opt/trn_rl_repo")
import numpy as np
import ml_dtypes
import concourse.bass as bass
import concourse.bacc as bacc
import concourse.mybir as mybir
from concourse.tile import TileContext
from concourse.bass_utils import run_bass_kernel_spmd

F32 = mybir.dt.float32
F32R = mybir.dt.float32r
BF16 = mybir.dt.bfloat16
OP = mybir.AluOpType
AF = mybir.ActivationFunctionType

B_, L, DM = 4, 2048, 1024       # batch, seqlen, d_model
DI = 2048                        # d_inner (global)
DL = 1024                        # d_inner per core
N = 16                           # d_state
RK = 64                          # dt_rank
KC = 4                           # conv width
TC = 512                         # time chunk (scan + out_proj stage)
NCH = L // TC                    # 4 chunks
NJ = DL // 128                   # 8 d-tiles per core
NK = DM // 128                   # 8 k-tiles over d_model
NM = DM // 128                   # 8 out d_model tiles
TP = TC + 1                      # scan segment length (flush col + TC)
NS = N + 2                       # y-scan slots: flush + 16 n + uD
PAIRS = [[0, 1], [2, 3], [4, 5], [6, 7]]

_CACHED_NC = {}


def _build(reps=1):
    nc = bacc.Bacc(num_devices=8)

    # ---- parameters (per-core shards) ----
    hst = nc.declare_dram_parameter("hst", [DM, L], F32, isOutput=False)
    wx = nc.declare_dram_parameter("wx", [DM, DL], F32, isOutput=False)
    wz = nc.declare_dram_parameter("wz", [DM, DL], F32, isOutput=False)
    wo = nc.declare_dram_parameter("wo", [DL, DM], F32, isOutput=False)
    wxp = nc.declare_dram_parameter("wxp", [DL, RK + 2 * N], F32, isOutput=False)
    wdt = nc.declare_dram_parameter("wdt", [RK, DL], F32, isOutput=False)
    convw = nc.declare_dram_parameter("convw", [128, NJ * KC], F32, isOutput=False)
    cbd = nc.declare_dram_parameter("cbd", [128, 3 * NJ], F32, isOutput=False)
    negA = nc.declare_dram_parameter("negA", [128, NJ * N], F32, isOutput=False)
    oslab = nc.declare_dram_parameter("oslab", [DM // 2, L], BF16, isOutput=True)

    with TileContext(nc) as tc:
        with tc.tile_pool(name="const", bufs=1) as cp:
            convw_t = cp.tile([128, NJ, KC], F32, tag="convw", name="convw_t")
            nc.sync.dma_start(
                out=convw_t[:, :, :],
                in_=convw[:, :].rearrange("p (j k) -> p j k", j=NJ))
            cbd_t = cp.tile([128, 3 * NJ], F32, tag="cbd", name="cbd_t")
            nc.sync.dma_start(out=cbd_t[:, :], in_=cbd[:, :])
            negA_t = cp.tile([128, NJ, N], F32, tag="negA", name="negA_t")
            nc.sync.dma_start(
                out=negA_t[:, :, :],
                in_=negA[:, :].rearrange("p (j n) -> p j n", j=NJ))
            wxp_t = cp.tile([128, NJ, RK + 2 * N], F32R, tag="wxp", name="wxp_t")
            nc.sync.dma_start(
                out=wxp_t[:, :, :],
                in_=wxp[:, :].rearrange("(j p) w -> p j w", j=NJ).bitcast(F32R))
            wdt_t = cp.tile([RK, DL], F32R, tag="wdt", name="wdt_t")
            nc.sync.dma_start(out=wdt_t[:, :], in_=wdt[:, :].bitcast(F32R))
            ones_t = cp.tile([128, TC, NS], BF16, tag="ones", name="ones_t")
            nc.vector.memset(ones_t[:, :, :], 1.0)
            nc.vector.memset(ones_t[:, :, 0], 0.0)
            carry = cp.tile([128, NJ, N], F32, tag="carry", name="carry_t")
            ubf = cp.tile([128, NJ, L], BF16, tag="ubf", name="ubf_t")

            for rep in range(reps):
                xdbl_in = nc.dram_tensor(f"xdbl_in{rep}", [RK + 2 * N, L], F32)
                xdbl_out = nc.dram_tensor(f"xdbl_out{rep}", [RK + 2 * N, L], F32)
                bc_dram = nc.dram_tensor(f"bc_dram{rep}", [2 * N, L], BF16)
                g_dram = nc.dram_tensor(f"g_dram{rep}", [DL, L], BF16)
                oc_in = nc.dram_tensor(f"oc_in{rep}", [DM, L], BF16)
                oc_out = nc.dram_tensor(f"oc_out{rep}", [DM // 2, L], BF16)

                with tc.tile_pool(name=f"hsp{rep}", bufs=1) as hp:
                    hsT = hp.tile([128, NK, L], F32R, tag="hsT", name="hsT")
                    nc.sync.dma_start(
                        out=hsT[:, :, :],
                        in_=hst[:, :].rearrange("(k p) t -> p k t",
                                                k=NK).bitcast(F32R))
                    _emit_in_x(nc, tc, rep, wx, hsT, convw_t, cbd_t, wxp_t,
                               ubf, xdbl_in)
                    nc.gpsimd.collective_compute(
                        "AllReduce", OP.add, replica_groups=PAIRS,
                        ins=[xdbl_in[:, :]], outs=[xdbl_out[:, :]])
                    _emit_in_z(nc, tc, rep, wz, hsT, g_dram)
                    # B/C rows -> bf16 dram for broadcast; dtraw -> clipped
                    with tc.tile_pool(name=f"bc{rep}", bufs=1) as bp:
                        bcf = bp.tile([2 * N, L], F32, tag="bcf", name="bcf")
                        nc.sync.dma_start(out=bcf[:, :],
                                          in_=xdbl_out[RK:RK + 2 * N, :])
                        bcb = bp.tile([2 * N, L], BF16, tag="bcb", name="bcb")
                        nc.vector.tensor_copy(bcb[:, :], bcf[:, :])
                        nc.sync.dma_start(out=bc_dram[:, :], in_=bcb[:, :])

                _emit_scan(nc, tc, rep, cbd_t, negA_t, wdt_t, wo, ones_t,
                           carry, ubf, g_dram, xdbl_out, bc_dram, oc_in)
                nc.gpsimd.collective_compute(
                    "ReduceScatter", OP.add, replica_groups=PAIRS,
                    ins=[oc_in[:, :]], outs=[oc_out[:, :]])
                nc.gpsimd.dma_start(out=oslab[:, :], in_=oc_out[:, :])

    nc.finalize()
    return nc


def _emit_in_x(nc, tc, rep, wx, hsT, convw_t, cbd_t, wxp_t, ubf, xdbl_in):
    """in_proj x-side + conv + silu + clip + x_proj partials."""
    with (
        tc.tile_pool(name=f"wxp{rep}", bufs=1) as wp,
        tc.tile_pool(name=f"xwork{rep}", bufs=1) as xw,
        tc.tile_pool(name=f"psx{rep}", bufs=1, space="PSUM") as psx,
        tc.tile_pool(name=f"psg{rep}", bufs=1, space="PSUM") as psg,
    ):
        wxt = wp.tile([128, NJ, NK, 128], F32R, tag="wx", name="wxt")
        nc.sync.dma_start(
            out=wxt[:, :, :, :],
            in_=wx[:, :].rearrange("(k p) (j q) -> p j k q",
                                   k=NK, j=NJ).bitcast(F32R))
        psxs = [psx.tile([RK + 2 * N, TC], F32, tag=f"psx{q}",
                         name=f"psx{q}") for q in range(4)]
        for j in range(NJ):
            xps = [psg.tile([128, TC], F32, tag=f"xps{q}", name=f"xps{q}")
                   for q in range(4)]
            for k in range(NK):
                for q in range(4):
                    nc.tensor.matmul(
                        xps[q][:, :], wxt[:, j, k, :],
                        hsT[:, k, q * TC:(q + 1) * TC],
                        start=(k == 0), stop=(k == NK - 1))
            xcj = xw.tile([128, KC - 1 + L], BF16, tag="xcj", name="xcj")
            nc.vector.memset(xcj[:, 0:KC - 1], 0.0)
            for q in range(4):
                nc.vector.tensor_scalar(
                    xcj[:, KC - 1 + q * TC:KC - 1 + (q + 1) * TC],
                    xps[q][:, :], 0.0, 1.0, op0=OP.max, op1=OP.min)
            cv = xw.tile([128, L], F32, tag="cv", name="cv")
            tmp = xw.tile([128, L], F32, tag="tmp", name="tmp")
            nc.vector.tensor_tensor(
                out=cv[:, :], in0=xcj[:, 0:L],
                in1=convw_t[:, j, 0:1].broadcast_to([128, L]), op=OP.mult)
            for k in range(1, KC):
                nc.vector.tensor_tensor(
                    out=tmp[:, :], in0=xcj[:, k:k + L],
                    in1=convw_t[:, j, k:k + 1].broadcast_to([128, L]),
                    op=OP.mult)
                nc.vector.tensor_tensor(out=cv[:, :], in0=cv[:, :],
                                        in1=tmp[:, :], op=OP.add)
            nc.scalar.activation(tmp[:, :], cv[:, :], AF.Silu,
                                 bias=cbd_t[:, j:j + 1])
            uf = xw.tile([128, L], F32R, tag="uf", name="uf")
            nc.vector.tensor_scalar(uf[:, :], tmp[:, :], 0.0, 1.0,
                                    op0=OP.max, op1=OP.min)
            nc.vector.tensor_copy(ubf[:, j, :], uf[:, :].bitcast(F32))
            for q in range(4):
                nc.tensor.matmul(
                    psxs[q][:, :], wxp_t[:, j, :],
                    uf[:, q * TC:(q + 1) * TC],
                    start=(j == 0), stop=(j == NJ - 1))
        xdbl = xw.tile([RK + 2 * N, L], F32, tag="xdbl", name="xdbl")
        for q in range(4):
            nc.scalar.copy(xdbl[:, q * TC:(q + 1) * TC], psxs[q][:, :])
        nc.sync.dma_start(out=xdbl_in[:, :], in_=xdbl[:, :])


def _emit_in_z(nc, tc, rep, wz, hsT, g_dram):
    """in_proj z-side + clip + silu -> gate (overlaps the AllReduce)."""
    with (
        tc.tile_pool(name=f"wzp{rep}", bufs=1) as wp,
        tc.tile_pool(name=f"zwork{rep}", bufs=1) as zw,
        tc.tile_pool(name=f"psz{rep}", bufs=1, space="PSUM") as psz,
    ):
        wzt = wp.tile([128, NJ, NK, 128], F32R, tag="wz", name="wzt")
        nc.sync.dma_start(
            out=wzt[:, :, :, :],
            in_=wz[:, :].rearrange(
                "(k p) (j q) -> p j k q", k=NK, j=NJ).bitcast(F32R))
        for j in range(NJ):
            zps = [psz.tile([128, TC], F32, tag=f"zps{q}", name=f"zps{q}")
                   for q in range(4)]
            for k in range(NK):
                for q in range(4):
                    nc.tensor.matmul(
                        zps[q][:, :], wzt[:, j, k, :],
                        hsT[:, k, q * TC:(q + 1) * TC],
                        start=(k == 0), stop=(k == NK - 1))
            zb = zw.tile([128, L], BF16, tag="zb", name="zb")
            for q in range(4):
                nc.vector.tensor_scalar(
                    zb[:, q * TC:(q + 1) * TC], zps[q][:, :],
                    0.0, 1.0, op0=OP.max, op1=OP.min)
            gt = zw.tile([128, L], BF16, tag="gt", name="gt")
            nc.scalar.activation(gt[:, :], zb[:, :], AF.Silu)
            nc.sync.dma_start(out=g_dram[j * 128:(j + 1) * 128, :],
                              in_=gt[:, :])


def _emit_scan(nc, tc, rep, cbd_t, negA_t, wdt_t, wo, ones_t, carry,
               ubf, g_dram, xdbl_out, bc_dram, oc_in):
    """dt path + selective scan + gate + out_proj partials, per chunk."""
    with (
        tc.tile_pool(name=f"chk{rep}", bufs=1) as ck,
        tc.tile_pool(name=f"jw{rep}", bufs=1) as jw,
        tc.tile_pool(name=f"sm{rep}", bufs=1) as sm,
        tc.tile_pool(name=f"pdt{rep}", bufs=2, space="PSUM") as pdt,
        tc.tile_pool(name=f"pop{rep}", bufs=2, space="PSUM") as pop,
    ):
        an = jw.tile([128, N, TP], F32, tag="an", name="an")
        bt = jw.tile([128, N, TP], BF16, tag="bt", name="bt")
        ch = jw.tile([128, TC, NS], BF16, tag="ch", name="ch")
        yg = jw.tile([128, NJ, TC], F32R, tag="yg", name="yg")
        nc.vector.memset(ch[:, :, 0], 0.0)
        for c in range(NCH):
            csl = slice(c * TC, (c + 1) * TC)
            drf = ck.tile([RK, TC], F32, tag="drf", name="drf")
            nc.sync.dma_start(out=drf[:, :], in_=xdbl_out[0:RK, csl])
            dtraw = ck.tile([RK, TC], F32R, tag="dtraw", name="dtraw")
            nc.vector.tensor_scalar(dtraw[:, :], drf[:, :], 0.0, 1.0,
                                    op0=OP.max, op1=OP.min)
            bbc = ck.tile([128, N, TC], BF16, tag="bbc", name="bbc")
            nc.sync.dma_start(
                out=bbc[:, :, :],
                in_=bc_dram[0:N, csl][None, :, :].broadcast_to([128, N, TC]))
            cbc = ck.tile([128, N, TC], BF16, tag="cbc", name="cbc")
            nc.sync.dma_start(
                out=cbc[:, :, :],
                in_=bc_dram[N:2 * N, csl][None, :, :].broadcast_to(
                    [128, N, TC]))
            for j in range(NJ):
                dps = pdt.tile([128, TC], F32, tag="dps", name="dps")
                nc.tensor.matmul(dps[:, :], wdt_t[:, j * 128:(j + 1) * 128],
                                 dtraw[:, :], start=True, stop=True)
                spe = sm.tile([128, TC], F32, tag="spe", name="spe")
                nc.scalar.activation(spe[:, :], dps[:, :], AF.Exp,
                                     bias=cbd_t[:, NJ + j:NJ + j + 1])
                dt = sm.tile([128, TC], F32, tag="dt", name="dt")
                nc.scalar.activation(dt[:, :], spe[:, :], AF.Ln, bias=1.0)
                # an[:, n, 1+t] = exp(negA[n] * dt[t]); col 0 stays 0 (flush)
                nc.vector.memset(an[:, :, 0], 0.0)
                nc.vector.tensor_tensor(
                    out=an[:, :, 1:],
                    in0=dt[:, None, :].broadcast_to([128, N, TC]),
                    in1=negA_t[:, j, :, None].broadcast_to([128, N, TC]),
                    op=OP.mult)
                nc.scalar.activation(an[:, :, 1:], an[:, :, 1:], AF.Exp)
                # carry inject into flush col of bt
                if c == 0:
                    nc.vector.memset(bt[:, :, 0], 0.0)
                else:
                    nc.vector.tensor_copy(bt[:, :, 0], carry[:, j, :])
                dtu = sm.tile([128, TC], BF16, tag="dtu", name="dtu")
                nc.vector.tensor_tensor(out=dtu[:, :], in0=dt[:, :],
                                        in1=ubf[:, j, csl], op=OP.mult)
                nc.vector.tensor_tensor(
                    out=bt[:, :, 1:],
                    in0=dtu[:, None, :].broadcast_to([128, N, TC]),
                    in1=bbc[:, :, :], op=OP.mult)
                # fused 16-state scan; in-place (ht := an)
                nc.vector.tensor_tensor_scan(
                    an[:, :, :].rearrange("p n t -> p (n t)"),
                    an[:, :, :].rearrange("p n t -> p (n t)"),
                    bt[:, :, :].rearrange("p n t -> p (n t)"), 0.0,
                    op0=OP.mult, op1=OP.add)
                if c < NCH - 1:
                    nc.vector.tensor_copy(carry[:, j, :], an[:, :, TP - 1])
                # ch[t, 1+n] = h * C ; ch[t, 17] = u * D ; col 0 stays 0
                nc.vector.tensor_tensor(
                    out=ch[:, :, 1:N + 1].transpose([0, 2, 1]),
                    in0=an[:, :, 1:], in1=cbc[:, :, :], op=OP.mult)
                nc.vector.tensor_tensor(
                    out=ch[:, :, N + 1], in0=ubf[:, j, csl],
                    in1=cbd_t[:, 2 * NJ + j:2 * NJ + j + 1].broadcast_to(
                        [128, TC]), op=OP.mult)
                # y-scan: sum over n slots (in-place over ch)
                nc.vector.tensor_tensor_scan(
                    ch[:, :, :].rearrange("p t n -> p (t n)"),
                    ones_t[:, :, :].rearrange("p t n -> p (t n)"),
                    ch[:, :, :].rearrange("p t n -> p (t n)"), 0.0,
                    op0=OP.mult, op1=OP.add)
                yt = sm.tile([128, TC], BF16, tag="yt", name="yt")
                nc.vector.tensor_scalar(yt[:, :], ch[:, :, N + 1],
                                        0.0, 1.0, op0=OP.max, op1=OP.min)
                grb = sm.tile([128, TC], BF16, tag="grb", name="grb")
                nc.scalar.dma_start(
                    out=grb[:, :],
                    in_=g_dram[j * 128:(j + 1) * 128, csl])
                nc.vector.tensor_tensor(out=yg[:, j, :],
                                        in0=yt[:, :], in1=grb[:, :],
                                        op=OP.mult)
            # out_proj for this chunk (weights streamed per m-tile)
            osb = sm.tile([128, NM, TC], BF16, tag="osb", name="osb")
            for m in range(NM):
                wom = sm.tile([128, NJ, 128], F32R, tag="wom", name="wom")
                nc.sync.dma_start(
                    out=wom[:, :, :],
                    in_=wo[:, m * 128:(m + 1) * 128].rearrange(
                        "(j p) q -> p j q", j=NJ).bitcast(F32R))
                ops = pop.tile([128, TC], F32, tag="ops", name="ops")
                for j in range(NJ):
                    nc.tensor.matmul(ops[:, :], wom[:, j, :],
                                     yg[:, j, :], start=(j == 0),
                                     stop=(j == NJ - 1))
                nc.scalar.copy(osb[:, m, :], ops[:, :])
            nc.sync.dma_start(
                out=oc_in[:, csl].rearrange("(m p) t -> p m t", m=NM),
                in_=osb[:, :, :])


def _shard(inputs):
    hs = np.asarray(inputs["hidden_states"], np.float32)
    W_in = np.asarray(inputs["W_in"], np.float32)
    conv_w = np.asarray(inputs["conv_w"], np.float32)
    conv_b = np.asarray(inputs["conv_b"], np.float32)
    W_x = np.asarray(inputs["W_x"], np.float32)
    W_dt = np.asarray(inputs["W_dt"], np.float32)
    b_dt = np.asarray(inputs["b_dt"], np.float32)
    W_out = np.asarray(inputs["W_out"], np.float32)
    A_log = np.asarray(inputs["A_log"], np.float32)
    D = np.asarray(inputs["D"], np.float32)

    in_maps = []
    for c in range(8):
        b, dh = c // 2, c % 2
        dsl = slice(dh * DL, (dh + 1) * DL)
        convw_l = np.ascontiguousarray(
            conv_w[dsl, 0, :].reshape(NJ, 128, KC).transpose(1, 0, 2)
            .reshape(128, NJ * KC))
        cbd_l = np.concatenate([
            conv_b[dsl].reshape(NJ, 128).T,
            b_dt[dsl].reshape(NJ, 128).T,
            D[dsl].reshape(NJ, 128).T], axis=1)
        negA_l = np.ascontiguousarray(
            (-np.exp(A_log[dsl])).reshape(NJ, 128, N).transpose(1, 0, 2)
            .reshape(128, NJ * N))
        m = {
            "hst": np.ascontiguousarray(hs[b].T),
            "wx": np.ascontiguousarray(W_in[dsl].T),
            "wz": np.ascontiguousarray(
                W_in[DI + dh * DL: DI + (dh + 1) * DL].T),
            "wo": np.ascontiguousarray(W_out[:, dsl].T),
            "wxp": np.ascontiguousarray(W_x[:, dsl].T),
            "wdt": np.ascontiguousarray(W_dt[dsl].T),
            "convw": convw_l,
            "cbd": np.ascontiguousarray(cbd_l),
            "negA": negA_l,
        }
        in_maps.append(m)
    return in_maps


def kernel(**inputs):
    if 1 not in _CACHED_NC:
        _CACHED_NC[1] = _build(1)
    nc = _CACHED_NC[1]
    in_maps = _shard(inputs)
    res = run_bass_kernel_spmd(nc, in_maps, core_ids=list(range(8)))
    out = np.empty((B_, L, DM), np.float32)
    for b in range(B_):
        s0 = np.asarray(res.results[2 * b]["oslab"], dtype=np.float32)
        s1 = np.asarray(res.results[2 * b + 1]["oslab"], dtype=np.float32)
        out[b] = np.concatenate([s0, s1], axis=0).T
    return out
